# revision 10
# baseline (speedup 1.0000x reference)
"""Multi-head attention Trainium2 Bass kernel (v2 — restructured).

Problem: B=4, N=M=2048, DM=512, H=8, DH=64, DO=512, fp32.
Sharding: 8 cores = (batch b, row-half) -- each core computes full attention
for 1024 query rows of one batch. No collectives.

v2 design vs baseline (264046 ns -> 180707 ns):
  - Q/K/V pre-transposed host-side (qT/kT/vT [DM, n]): no PE transposes and
    no transpose-copy traffic on device.
  - exp split between ScalarE (exact LUT, bf16 out) and VectorE (Schraudolph
    bitcast approx: one tensor_scalar into an int16 view of a bf16 tile), so
    the attention inner loop is PE-bound instead of ScalarE-bound.
  - single flat software pipeline over all 64 (block, mu) iterations: the oh
    matmuls of iteration i-1 are emitted at iteration i, normalization of
    block b at block b+1, so the PE never drains at block boundaries.
  - K-proj(ms1..3), V-proj, and Q-proj(ns1) stream inside attention blocks
    0-1 so the whole 13MB input DMA hides under compute; dc-outer projection
    loops open 4 PSUM accumulation groups at once.
  - PSUM: 6 banks rotate (3x [128,1024]) between scores, V pairs, and the
    output projection; 2 banks hold the oh accumulators. Warm-up matmuls on
    the oh banks keep the PE p-state warm through DMA-paced stretches.
  - softmax normalization: denominator row to partition 0 (shifted DVE copy;
    custom DVE ops only lower correctly at base partition 0), approximate
    reciprocal on DVE, partition-broadcast on GpSimd, multiply on DVE.
  - v_bias folded into the projection bias host-side; output projection does
    head PAIRS per matmul plus a ones-row bias matmul, staged out via
    alternating ScalarE/VectorE copies.
"""
import os
import sys

sys.path.insert(0, "/opt/trn_rl_repo")

import numpy as np

import concourse.bass as bass
import concourse.mybir as mybir
import concourse.tile as tile
from concourse import bacc
from concourse.bass_utils import run_bass_kernel_spmd

F32 = mybir.dt.float32
F32R = mybir.dt.float32r
BF16 = mybir.dt.bfloat16
I16 = mybir.dt.int16
EXP = mybir.ActivationFunctionType.Exp
ADD = mybir.AluOpType.add
MULT = mybir.AluOpType.mult

P = 128
DM = 512
HDH = 512
DH = 64
H = 8
NB = 1024     # query rows per core
M = 2048      # kv rows
DO = 512
N_MT = M // P
N_QT = NB // P

# Schraudolph exp in the bf16 bit domain: bitcast(int16(x*A + B)) ~= e^x,
# max rel err ~3% (walrus requires f32r matmul inputs to be f32r-rounded, so
# the approx-exp tiles are bf16 instead)
SCH_A = float(128.0 / np.log(2.0))
SCH_B = float(127.0 * 128.0 - 5.0)
# which (2*mu + ab) slots compute exp on DVE (7/16; rest on ScalarE)
DVE_EXP_SLOTS = (1, 3, 7, 11, 13, 15)

_CACHED = {}
LAST_EXEC_NS = None


def _build():
    nc = bacc.Bacc("TRN2", target_bir_lowering=False, debug=False)

    d_qT = nc.declare_dram_parameter("qT", [DM, NB], F32R, isOutput=False)
    d_kT = nc.declare_dram_parameter("kT", [DM, M], F32R, isOutput=False)
    d_vT = nc.declare_dram_parameter("vT", [DM, M], F32R, isOutput=False)
    d_wq = nc.declare_dram_parameter("wq", [DM, HDH], F32R, isOutput=False)
    d_wk = nc.declare_dram_parameter("wk", [DM, HDH], F32R, isOutput=False)
    d_wv = nc.declare_dram_parameter("wv", [DM, HDH], F32R, isOutput=False)
    d_wp = nc.declare_dram_parameter("wp", [HDH, DO], F32R, isOutput=False)
    d_qkb = nc.declare_dram_parameter("qkb", [P, 8], F32, isOutput=False)
    d_pb = nc.declare_dram_parameter("pb", [1, DO], BF16, isOutput=False)
    d_ones = nc.declare_dram_parameter("ones", [1, 64], F32R, isOutput=False)
    d_out = nc.declare_dram_parameter("out", [NB, DO], F32, isOutput=True)

    # DRAM-side [dm, x] -> SBUF [p, dc, x] gather views
    def dram3(d, x):
        return d.rearrange("(dc p) x -> p dc x", p=P)

    with tile.TileContext(nc) as tc:
        from contextlib import ExitStack
        with ExitStack() as ctx:
            persist = ctx.enter_context(tc.tile_pool(name="persist", bufs=1))
            wts = ctx.enter_context(tc.tile_pool(name="wts", bufs=1))
            raw = ctx.enter_context(tc.tile_pool(name="raw", bufs=1))
            exp_pool = ctx.enter_context(tc.tile_pool(name="expp", bufs=5))
            nm = ctx.enter_context(tc.tile_pool(name="nm", bufs=2))
            # 6 banks rotate between scores, V-projection pairs and the
            # output projection; 2 banks hold the two oh accumulators
            ps_big = ctx.enter_context(tc.tile_pool(name="ps_big", bufs=3, space="PSUM"))
            ps_oh = ctx.enter_context(tc.tile_pool(name="ps_oh", bufs=1, space="PSUM"))

            # --- PE warm-up: dummy matmuls on a memset tile keep the PE busy
            # (and its p-state ramp warm) while the first DMAs land
            scratch = wts.tile([P, 512], BF16, tag="scratch", name="scratch")
            nc.vector.memset(scratch[:], 1.0)
            def emit_warmup(n):
                # filler matmuls target the (not yet used) oh banks so they
                # never perturb the big-pool rotation
                wm = ps_oh.tile([P, 512], F32, tag="oh0", name="wm")
                for i in range(n):
                    nc.tensor.matmul(
                        wm[:], scratch[:, 0:P], scratch[:],
                        start=True, stop=True, skip_group_check=True,
                    )
            emit_warmup(12)

            # --- Q first (smallest stream): attention can start earliest
            wqB = wts.tile([P, 4, HDH], F32R, tag="wqB", name="wqB")
            nc.sync.dma_start(wqB[:], dram3(d_wq, HDH))
            qrB = raw.tile([P, 4, NB], F32R, tag="qr", name="qrB")
            nc.sync.dma_start(qrB[:, :, 0:512], dram3(d_qT, NB)[:, :, 0:512])
            # K: wk and kT(ms0) interleaved per dm-chunk, then ms1
            wk_sb = [wts.tile([P, HDH], F32R, tag=f"wk{dc}", name=f"wk{dc}")
                     for dc in range(4)]
            krB = raw.tile([P, 4, M], F32R, tag="kr", name="krB")
            for dc in range(4):
                nc.sync.dma_start(wk_sb[dc][:], d_wk[dc * P:(dc + 1) * P, :])
                nc.sync.dma_start(
                    krB[:, dc, 0:512], d_kT[dc * P:(dc + 1) * P, 0:512])
            nc.sync.dma_start(
                krB[:, :, 512:1024], dram3(d_kT, M)[:, :, 512:1024])
            # small constants (needed from the first bias add onward)
            qkb = persist.tile([P, 8], F32, tag="qkb", name="qkb")
            nc.sync.dma_start(qkb[:], d_qkb[:])
            qb = qkb[:, 0:4]
            kb = qkb[:, 4:8]
            pbs = persist.tile([1, DO], BF16, tag="pbs", name="pbs")
            nc.sync.dma_start(pbs[:], d_pb[:])
            # V weights + interleaved vT/kT streams (consumed inside block 0)
            wv_sb = [wts.tile([P, HDH], F32R, tag=f"wv{dc}", name=f"wv{dc}")
                     for dc in range(4)]
            for dc in range(4):
                nc.sync.dma_start(wv_sb[dc][:], d_wv[dc * P:(dc + 1) * P, :])
            vrB = raw.tile([P, 4, M], F32R, tag="vr", name="vrB")
            nc.sync.dma_start(vrB[:, :, 0:512], dram3(d_vT, M)[:, :, 0:512])
            for i in range(2, 4):
                nc.sync.dma_start(
                    krB[:, :, i * 512:(i + 1) * 512],
                    dram3(d_kT, M)[:, :, i * 512:(i + 1) * 512])
                nc.sync.dma_start(
                    vrB[:, :, (i - 1) * 512:i * 512],
                    dram3(d_vT, M)[:, :, (i - 1) * 512:i * 512])
            nc.sync.dma_start(vrB[:, :, 1536:2048], dram3(d_vT, M)[:, :, 1536:2048])
            # second qT half (needed from block 1 on) and projection weights
            nc.sync.dma_start(qrB[:, :, 512:1024], dram3(d_qT, NB)[:, :, 512:1024])
            wpB = persist.tile([P, 4, DO], F32R, tag="wpB", name="wpB")
            nc.sync.dma_start(wpB[:], dram3(d_wp, DO))
            ones = persist.tile([1, 64], F32R, tag="ones", name="ones")
            nc.sync.dma_start(ones[:], d_ones[:])

            # --- persistent activations ---
            kTf = [persist.tile([P, M], BF16, tag=f"kTf{i}", name=f"kTf{i}")
                   for i in range(4)]
            qTf = [persist.tile([P, NB], BF16, tag=f"qTf{i}", name=f"qTf{i}")
                   for i in range(4)]
            vha = persist.tile([P, N_MT, 8 * 65], BF16, tag="vha", name="vha")
            mh = [persist.tile([P, NB], F32R, tag=f"mh{p4}", name=f"mh{p4}")
                  for p4 in range(4)]

            # === projections (dc-outer, 4 open PSUM groups per chunk) ===
            def emit_k_proj(ms):
                ppA = ps_big.tile([P, 1024], F32, tag="big", name="ppA")
                ppB = ps_big.tile([P, 1024], F32, tag="big", name="ppB")
                pp = [ppA[:, 0:512], ppA[:, 512:1024],
                      ppB[:, 0:512], ppB[:, 512:1024]]
                for dc in range(4):
                    for ht in range(4):
                        nc.tensor.matmul(
                            pp[ht], wk_sb[dc][:, ht * P:(ht + 1) * P],
                            krB[:, dc, ms * 512:(ms + 1) * 512],
                            start=(dc == 0), stop=(dc == 3),
                            skip_group_check=True,
                        )
                for ht in range(4):
                    nc.vector.tensor_scalar(
                        kTf[ht][:, ms * 512:(ms + 1) * 512],
                        pp[ht], kb[:, ht:ht + 1], None, ADD,
                    )

            def emit_q_proj(ns, half=None):
                hts = range(4) if half is None else range(2 * half, 2 * half + 2)
                ppA = ps_big.tile([P, 1024], F32, tag="big", name="ppA")
                pp = {hts[0] if isinstance(hts, list) else list(hts)[0]: ppA[:, 0:512],
                      list(hts)[1]: ppA[:, 512:1024]}
                if half is None:
                    ppB = ps_big.tile([P, 1024], F32, tag="big", name="ppB")
                    pp = {0: ppA[:, 0:512], 1: ppA[:, 512:1024],
                          2: ppB[:, 0:512], 3: ppB[:, 512:1024]}
                else:
                    pp = {list(hts)[0]: ppA[:, 0:512], list(hts)[1]: ppA[:, 512:1024]}
                for dc in range(4):
                    for ht in hts:
                        nc.tensor.matmul(
                            pp[ht], wqB[:, dc, ht * P:(ht + 1) * P],
                            qrB[:, dc, ns * 512:(ns + 1) * 512],
                            start=(dc == 0), stop=(dc == 3),
                            skip_group_check=True,
                        )
                for ht in hts:
                    nc.vector.tensor_scalar(
                        qTf[ht][:, ns * 512:(ns + 1) * 512],
                        pp[ht], qb[:, ht:ht + 1], None, ADD,
                    )

            emit_q_proj(0)
            emit_warmup(10)
            emit_k_proj(0)
            emit_warmup(20)
            # === V projection (streamed into attention block 0) ===
            # ones column of vh_aug (col 64 of each head group)
            nc.vector.tensor_copy(
                vha.rearrange("p a (h c) -> p a h c", c=65)[:, :, :, 64:65],
                scratch[:, 0:N_MT * 8].rearrange(
                    "p (a h) -> p a h", a=N_MT)[:, :, :, None],
            )

            def emit_v_pair(mp):
                """Project V m-tiles 2mp, 2mp+1 into vha."""
                vpp = ps_big.tile([P, 1024], F32, tag="big", name="vpp")
                for half in range(2):
                    mt = 2 * mp + half
                    for dc in range(4):
                        nc.tensor.matmul(
                            vpp[:, half * 512:(half + 1) * 512],
                            vrB[:, dc, mt * P:(mt + 1) * P], wv_sb[dc][:],
                            start=(dc == 0), stop=(dc == 3),
                            skip_group_check=True,
                        )
                for half in range(2):
                    mt = 2 * mp + half
                    dst = vha.rearrange("p a (h c) -> p a h c",
                                        c=65)[:, mt, :, 0:64]
                    src = vpp[:, half * 512:(half + 1) * 512].rearrange(
                        "p (h c) -> p h c", h=H)
                    if half == 0:
                        nc.scalar.copy(dst, src)
                    else:
                        nc.vector.tensor_copy(dst, src)

            # === output projection groups (head pairs) ===
            def emit_out_group(nt, tail):
                po_t = ps_big.tile([P, 1024], F32, tag="big", name="po")
                po = po_t[:, 0:DO]
                # bias matmul first: it has no mh dependency, so only the
                # final head-pair matmul waits on the last normalization
                nc.tensor.matmul(
                    po[:], scratch[0:1, 0:P], pbs[:],
                    start=True, stop=False, skip_group_check=True,
                )
                for p4 in range(4):
                    nc.tensor.matmul(
                        po[:], mh[p4][:, nt * P:(nt + 1) * P], wpB[:, p4, :],
                        start=False, stop=(p4 == 3),
                        skip_group_check=True,
                    )
                ot = nm.tile([P, DO], F32, tag="ot", name="ot", bufs=4)
                if nt % 2 == 0:
                    nc.scalar.copy(ot[:], po[:])
                else:
                    nc.vector.tensor_copy(ot[:], po[:])
                nc.sync.dma_start(d_out[nt * P:(nt + 1) * P, :], ot[:])

            # === Phase B: attention ===
            from concourse.dve_ops import (
                RECIP_APPROX_FAST_CONSTS, RECIPROCAL_APPROX_FAST)
            _rc = RECIP_APPROX_FAST_CONSTS

            def emit_norm(blk, oh, last, only_ab=None):
                """mh[hp][64*ab:64*ab+64, nb-half] = oh[ab] / den"""
                nb, hp = divmod(blk, 4)
                ns = slice(nb * 512, (nb + 1) * 512)
                for ab in range(2):
                    if only_ab is not None and ab != only_ab:
                        continue
                    # denominator row to partition 0 (custom-dve ops only
                    # lower correctly at base partition 0)
                    den0 = nm.tile([1, 512], F32, tag="den0", name="den0", bufs=1)
                    nc.vector.tensor_copy(den0[0:1, :], oh[ab][64:65, :])
                    ohsb = nm.tile([64, 512], F32, tag="ohsb", name="ohsb")
                    nc.scalar.copy(ohsb[:], oh[ab][0:64, :])
                    rr0 = nm.tile([1, 512], F32R if last else F32,
                                  tag="rrr" if last else "rr0", name="rr0",
                                  bufs=2 if not last else 1)
                    nc.vector._custom_dve(
                        RECIPROCAL_APPROX_FAST,
                        out=rr0[0:1, :], in0=den0[0:1, :],
                        s0=_rc["s0"], s1=_rc["s1"], imm2=_rc["imm2"],
                    )
                    if last:
                        # tail-critical: broadcast via a PE ones-matmul into
                        # the just-freed oh bank instead of the slower Pool op
                        bc_ps = ps_oh.tile([64, 512], F32, tag=f"oh{ab}",
                                           name="bc_ps")
                        nc.tensor.matmul(
                            bc_ps[:], ones[0:1, :],
                            rr0[0:1, :],
                            start=True, stop=True, skip_group_check=True,
                        )
                        nc.vector.tensor_tensor(
                            mh[hp][64 * ab:64 * ab + 64, ns],
                            ohsb[:], bc_ps[:], MULT,
                        )
                    else:
                        bc = nm.tile([64, 512], F32, tag="bc", name="bc")
                        nc.gpsimd.partition_broadcast(bc[:], rr0[0:1, :])
                        nc.vector.tensor_tensor(
                            mh[hp][64 * ab:64 * ab + 64, ns],
                            ohsb[:], bc[:], MULT,
                        )

            # Flat software pipeline over all 64 (block, mu) iterations:
            # iteration i emits scores+exp(i) and the oh matmuls of i-1, so
            # the PE never drains across block boundaries.
            oh_of = {}
            pend = None  # (blk, mu, ex_dict) awaiting oh matmuls

            def emit_oh(blk, mu, ex):
                hp = blk % 4
                for ab in range(2):
                    h = 2 * hp + ab
                    for j in range(2):
                        mt = 2 * mu + j
                        nc.tensor.matmul(
                            oh_of[blk][ab][0:65, :],
                            vha[:, mt, h * 65:h * 65 + 65],
                            ex[ab][:, j * 512:(j + 1) * 512],
                            start=(mu == 0 and j == 0),
                            stop=(mu == 7 and j == 1),
                        )

            for it in range(64):
                blk, mu = divmod(it, 8)
                nb, hp = divmod(blk, 4)
                ns = slice(nb * 512, (nb + 1) * 512)
                if mu == 0:
                    oh_of[blk] = {
                        ab: ps_oh.tile([P, 512], F32, tag=f"oh{ab}",
                                       name=f"oh{ab}")
                        for ab in range(2)
                    }
                ex_cur = {}
                for ab in range(2):
                    base = ab * 64
                    sc = ps_big.tile([P, 1024], F32, tag="big", name="sc")
                    for j in range(2):
                        mt = 2 * mu + j
                        nc.tensor.matmul(
                            sc[:, j * 512:(j + 1) * 512],
                            kTf[hp][base:base + 64, mt * P:(mt + 1) * P],
                            qTf[hp][base:base + 64, ns],
                            start=True, stop=True,
                            tile_position=(base, 0),
                        )
                    ex = exp_pool.tile([P, 1024], BF16, tag="ex", name="ex")
                    slot = (2 * mu + ab) % 16
                    if slot in DVE_EXP_SLOTS:
                        # Schraudolph approx exp on VectorE
                        nc.vector.tensor_scalar(
                            ex[:].bitcast(I16), sc[:],
                            SCH_A, SCH_B, MULT, ADD,
                        )
                    else:
                        nc.scalar.activation(ex[:], sc[:], EXP)
                    ex_cur[ab] = ex
                if blk == 0:
                    if mu in (0, 2, 4):
                        emit_k_proj(1 + mu // 2)
                    emit_v_pair(mu)
                if blk == 1 and mu in (1, 3):
                    emit_q_proj(1, half=mu // 2)
                if mu == 1 and blk >= 1:
                    # previous block's normalization, spread over two
                    # iterations so the copies don't bunch up on ACT/DVE
                    emit_norm(blk - 1, oh_of[blk - 1], last=False, only_ab=0)
                if mu == 2 and blk >= 1:
                    emit_norm(blk - 1, oh_of.pop(blk - 1), last=False, only_ab=1)
                if mu == 4 and nb == 1:
                    # nb0's output projection overlaps nb1's attention
                    emit_out_group(hp, tail=False)
                if pend is not None:
                    emit_oh(*pend)
                pend = (blk, mu, ex_cur)

            # === Phase C tail: drain + last norm + second n-half projection ===
            emit_oh(*pend)
            emit_norm(7, oh_of.pop(7), last=True)
            for nt in range(4, N_QT):
                emit_out_group(nt, tail=True)

    nc.compile()
    return nc


def kernel(query, key, value, query_kernel, key_kernel, value_kernel,
           projection_kernel, q_bias, k_bias, v_bias, projection_bias):
    query = np.asarray(query, dtype=np.float32)
    key = np.asarray(key, dtype=np.float32)
    value = np.asarray(value, dtype=np.float32)
    scale = np.float32(1.0 / 8.0)  # 1/sqrt(DH)

    wq = np.ascontiguousarray(
        (np.asarray(query_kernel, np.float32) * scale).transpose(1, 0, 2).reshape(DM, HDH))
    wk = np.ascontiguousarray(
        np.asarray(key_kernel, np.float32).transpose(1, 0, 2).reshape(DM, HDH))
    wv = np.ascontiguousarray(
        np.asarray(value_kernel, np.float32).transpose(1, 0, 2).reshape(DM, HDH))
    wp = np.ascontiguousarray(np.asarray(projection_kernel, np.float32).reshape(HDH, DO))
    qb = (np.asarray(q_bias, np.float32) * scale).reshape(HDH).reshape(4, P).T
    kb = np.asarray(k_bias, np.float32).reshape(HDH).reshape(4, P).T
    qkb = np.ascontiguousarray(np.concatenate([qb, kb], axis=1))
    # fold v_bias through the output projection into the projection bias:
    # out += sum_h vb[h] @ wp[h]
    vb = np.asarray(v_bias, np.float32)
    wp3 = np.asarray(projection_kernel, np.float32)
    import ml_dtypes
    pb = (np.asarray(projection_bias, np.float32)
          + np.einsum('hd,hdo->o', vb, wp3)).reshape(1, DO).astype(ml_dtypes.bfloat16)

    if "nc" not in _CACHED:
        _CACHED["nc"] = _build()
    nc = _CACHED["nc"]

    ones = np.ones((1, 64), dtype=np.float32)
    shared = dict(wq=wq, wk=wk, wv=wv, wp=wp, qkb=qkb, pb=pb, ones=ones)
    in_maps = []
    kT = [np.ascontiguousarray(key[b].T) for b in range(query.shape[0])]
    vT = [np.ascontiguousarray(value[b].T) for b in range(query.shape[0])]
    for c in range(8):
        b, half = c // 2, c % 2
        in_maps.append(dict(
            qT=np.ascontiguousarray(query[b, half * NB:(half + 1) * NB, :].T),
            kT=kT[b], vT=vT[b], **shared))

    trace = os.environ.get("KERNEL_TRACE", "0") == "1"
    try:
        res = run_bass_kernel_spmd(nc, in_maps, core_ids=list(range(8)), trace=trace)
    except ModuleNotFoundError:
        # axon NTFF profiling hook unavailable -- run without tracing
        res = run_bass_kernel_spmd(nc, in_maps, core_ids=list(range(8)), trace=False)
    global LAST_EXEC_NS
    LAST_EXEC_NS = res.exec_time_ns
    if trace and res.exec_time_ns is not None:
        print(f"HW exec time: {res.exec_time_ns} ns")
        if res.instructions_and_trace is not None:
            print(f"trace: {res.instructions_and_trace[1]}")

    B = query.shape[0]
    out = np.empty((B, 2 * NB, DO), dtype=np.float32)
    for c in range(8):
        b, half = c // 2, c % 2
        out[b, half * NB:(half + 1) * NB, :] = res.results[c]["out"]
    return out
